# revision 1
# baseline (speedup 1.0000x reference)
"""Trainium2 Bass kernel for the batched MPS quantum-circuit forward pass.

Math: every gate update in the reference circuit is local to one site, and the
CNOT MPO application is pure index bookkeeping (A_CTRL/B_TGT are 0/1 tensors).
Writing lam = (m0 m1 m2 m3) for the left-bond bits and rho = (a0 a1 a2 a3) for
the right-bond bits, the final site tensor factorizes in closed form:

  interior q:  T[q][lam, rho, i] = delta(i, a3) * prod_l U_l[a_l ^ m_l, a_{l-1}]
  site 0:      same with m = 0 (only lam = 0 nonzero)
  site 19:     T[19][lam, 0, i]  = sum_{a0 a1 a2} (same product), i = a3

with U_l = RZ(z_l) RY(y_l) the per-(batch, qubit, layer) 2x2 gate and
a_{-1} = 0.  So the kernel computes the four gate entry tables, the pairwise
chain products C01 = F0*F1 (16/site) and C23 = F2*F3 (32/site), then expands
out[lam, rho] = C01[m0 m1 a0 a1] * C23[m2 m3 a1 a2 a3] with gather-style
access patterns, writing straight into the interleaved complex64 layout.

Sharding: pure data parallelism - batch 1024 is split 128 per core across the
8 cores (partition dim = batch).
"""

import sys

sys.path.insert(0, "/opt/trn_rl_repo")

import numpy as np

B_TOTAL = 1024
N_CORES = 8
B = B_TOTAL // N_CORES  # 128 rows per core == SBUF partitions
NQ = 20
P_COLS = 160
ROW_F32 = NQ * 16 * 16 * 2 * 2  # 20480 fp32 per batch row (interleaved complex)

_CACHE = {}


def _build_nc():
    import concourse.bass as bass
    import concourse.tile as tile
    from concourse import bacc, mybir

    f32 = mybir.dt.float32
    MUL = mybir.AluOpType.mult
    SIN = mybir.ActivationFunctionType.Sin

    nc = bacc.Bacc("TRN2", target_bir_lowering=False, debug=False)
    theta_d = nc.dram_tensor("theta", [B, P_COLS], f32, kind="ExternalInput").ap()
    out_d = nc.dram_tensor("out", [B, ROW_F32], f32, kind="ExternalOutput").ap()

    from contextlib import ExitStack

    with tile.TileContext(nc) as tc, ExitStack() as ctx:
        pool = ctx.enter_context(tc.tile_pool(name="main", bufs=1))

        def tl(name, w):
            return pool.tile([B, w], f32, name=name)

        th = tl("th", 160)
        sinv = tl("sinv", 160)
        cosv = tl("cosv", 160)
        halfpi = tl("halfpi", 1)
        f16b = mybir.dt.float16
        p8 = pool.tile([B, 640], f16b, name="p8")          # zones of 80: cc sc cs ss -cc -sc -cs -ss
        f0 = tl("f0", 160)          # [m0,a0,q] re | im
        f1 = tl("f1", 320)          # [m1,a1,a0,q] re | im
        f2 = tl("f2", 320)          # [m2,a2,a1,q]
        f3 = tl("f3", 320)          # [m3,a3,a2,q]
        c01 = tl("c01", 640)        # per site 16: m0*8+m1*4+a0*2+a1 ; re | im
        c23 = tl("c23", 1280)       # per site 32: m2*16+m3*8+a1*4+a2*2+a3 ; re | im
        f16 = mybir.dt.float16
        c01q = pool.tile([B, 640], f16, name="c01q")   # col = idx*20+q (re|im)
        c23q = pool.tile([B, 1280], f16, name="c23q")
        ca = pool.tile([B, 320], f16b, name="ca")
        cb = pool.tile([B, 320], f16b, name="cb")
        cc_s = pool.tile([B, 640], f16b, name="cc_s")
        cd_s = pool.tile([B, 640], f16b, name="cd_s")
        ce_s = pool.tile([B, 640], f16b, name="ce_s")
        cf_s = pool.tile([B, 640], f16b, name="cf_s")
        cd1 = pool.tile([B, 320], f16b, name="cd1")
        cd2 = pool.tile([B, 320], f16b, name="cd2")
        t1 = pool.tile([B, 512], f16, name="t1")
        t2 = pool.tile([B, 512], f16, name="t2")
        t3 = pool.tile([B, 512], f16, name="t3")
        t4 = pool.tile([B, 512], f16, name="t4")
        tp1 = pool.tile([B, 512], f16, name="tp1")
        tp2 = pool.tile([B, 512], f16, name="tp2")
        tp3 = pool.tile([B, 512], f16, name="tp3")
        tp4 = pool.tile([B, 512], f16, name="tp4")
        s01 = tl("s01", 16)
        s02 = tl("s02", 16)
        s03 = tl("s03", 16)
        s04 = tl("s04", 16)
        u19a = tl("u19a", 256)
        u19b = tl("u19b", 256)
        pr19 = tl("pr19", 256)
        pi19 = tl("pi19", 256)
        r1r = tl("r1r", 128)
        r1i = tl("r1i", 128)
        r2r = tl("r2r", 64)
        r2i = tl("r2i", 64)
        sr = tl("sr", 32)
        si = tl("si", 32)
        outa = tl("outa", 7 * 1024)   # sites 0..6
        outb = tl("outb", 6 * 1024)   # sites 7..12
        outc = tl("outc", 6 * 1024)   # sites 13..18
        outd = tl("outd", 1024)       # site 19

        def ap(t, off, dims):
            w = t.shape[1]
            return bass.AP(tensor=t.tensor, offset=t.offset + off, ap=[[w, B]] + dims)

        # ---- stage A: angles -> sin/cos of half-angles --------------------
        nc.vector.memset(halfpi[:], float(np.pi / 2))
        warm = tl("warm", 1)
        nc.scalar.activation(warm[:], halfpi[:], SIN, scale=0.5)
        nc.sync.dma_start(th[:], theta_d)
        nc.scalar.activation(sinv[:], th[:], SIN, scale=0.5)
        # cos(x) = sin(pi/2 - |x|), keeps the Sin operand inside [-pi, pi]
        absv = tl("absv", 160)
        nc.scalar.activation(absv[:], th[:], mybir.ActivationFunctionType.Abs, scale=0.5)
        nc.scalar.activation(cosv[:], absv[:], SIN, bias=halfpi[:], scale=-1.0)

        # ---- stage B: base products p8 -----------------------------------
        # theta col = l*40 + g*20 + q ; g=0 -> RY(y), g=1 -> RZ(z)
        # zone z col = z*80 + l*20 + q
        # cc = cos(y/2)cos(z/2), sc = cos(y/2)sin(z/2),
        # cs = sin(y/2)cos(z/2), ss = sin(y/2)sin(z/2)
        lq = [[20, 4], [1, 20]]
        thlq = [[40, 4], [1, 20]]
        for zone, (g0, g1) in enumerate([(cosv, cosv), (cosv, sinv), (sinv, cosv), (sinv, sinv)]):
            nc.vector.tensor_tensor(
                ap(p8, zone * 80, lq), ap(g0, 0, thlq), ap(g1, 20, thlq), MUL
            )
        nc.vector.tensor_scalar_mul(ap(p8, 320, [[1, 320]]), ap(p8, 0, [[1, 320]]), -1.0)

        # ---- stages C/D/E: C01, C23 built straight from p8 ----------------
        # F_l[m,a,c] = U_l[a^m, c]: affine (base, c-stride) per parity a^m.
        Z = {"cc": 0, "sc": 80, "cs": 160, "ss": 240, "-sc": 400, "-cs": 480}
        F_RE = {0: (Z["cc"], Z["-cs"] - Z["cc"]), 1: (Z["cs"], Z["cc"] - Z["cs"])}
        F_IM = {0: (Z["-sc"], Z["ss"] - Z["-sc"]), 1: (Z["ss"], Z["sc"] - Z["ss"])}
        F0_RE = {0: (Z["cc"], Z["cs"] - Z["cc"]), 1: (Z["cs"], Z["cc"] - Z["cs"])}
        F0_IM = {0: (Z["-sc"], Z["ss"] - Z["-sc"]), 1: (Z["ss"], Z["-sc"] - Z["ss"])}
        PL01 = ((F0_RE, F_RE), (F0_IM, F_IM), (F0_RE, F_IM), (F0_IM, F_RE))
        PL23 = ((F_RE, F_RE), (F_IM, F_IM), (F_RE, F_IM), (F_IM, F_RE))

        # C01 = F0*F1: col q*16 + m0*8+m1*4+a0*2+a1 (re | im at +320)
        s01s = [ca, cb, cd1, cd2]
        k = 0
        for si_, (pl0, pl1) in enumerate(PL01):
            for m0 in (0, 1):
                b0, s0 = pl0[m0]
                for p1 in (0, 1):
                    b1, s1 = pl1[p1]
                    d1b, d1s = (0, 5) if p1 == 0 else (1, 3)
                    eng = (nc.vector, nc.vector, nc.vector, nc.gpsimd)[k % 4]
                    eng.tensor_tensor(
                        ap(s01s[si_], (m0 * 8 + d1b) * 20, [[d1s * 20, 2], [40, 2], [1, 20]]),
                        ap(p8, b0, [[0, 2], [s0, 2], [1, 20]]),
                        ap(p8, b1 + 20, [[0, 2], [s1, 2], [1, 20]]),
                        MUL,
                    )
                    k += 1
        cL = [[1, 320]]
        nc.vector.tensor_sub(ap(c01q, 0, cL), ap(ca, 0, cL), ap(cb, 0, cL))
        nc.vector.tensor_add(ap(c01q, 320, cL), ap(cd1, 0, cL), ap(cd2, 0, cL))

        # C23 = F2*F3: col q*32 + m2*16+m3*8+a1*4+a2*2+a3 (re | im at +640)
        s23s = [cc_s, cd_s, ce_s, cf_s]
        k = 0
        for si_, (pl2, pl3) in enumerate(PL23):
            for p2 in (0, 1):
                b2, s2 = pl2[p2]
                d2b, d2s = (0, 18) if p2 == 0 else (2, 14)
                for p3 in (0, 1):
                    b3, s3 = pl3[p3]
                    d3b, d3s = (0, 9) if p3 == 0 else (1, 7)
                    f3o = b3 + 60 + (s3 if p2 == 1 else 0)
                    f3s = s3 if p2 == 0 else -s3
                    for a1 in (0, 1):
                        eng = (nc.vector, nc.vector, nc.vector, nc.gpsimd)[k % 4]
                        eng.tensor_tensor(
                            ap(s23s[si_], (d2b + d3b + a1 * 4) * 20, [[d2s * 20, 2], [d3s * 20, 2], [1, 20]]),
                            ap(p8, b2 + 40 + a1 * s2, [[0, 2], [0, 2], [1, 20]]),
                            ap(p8, f3o, [[f3s, 2], [0, 2], [1, 20]]),
                            MUL,
                        )
                        k += 1
        eL = [[1, 640]]
        nc.vector.tensor_sub(ap(c23q, 0, eL), ap(cc_s, 0, eL), ap(cd_s, 0, eL))
        nc.vector.tensor_add(ap(c23q, 640, eL), ap(ce_s, 0, eL), ap(cf_s, 0, eL))

        # ---- hole zero-fill (positions that stay zero) --------------------
        # Broadcast-copied from a small zero tile on the otherwise-idle
        # Activation engine (frees ~10us of gpsimd time for stage F).
        # Gated past the trig chain so the greedy tile scheduler cannot
        # stuff these long copies in front of sin/cos on the Act queue.
        zq = tl("zq", 64)
        nc.vector.memset(zq[:], 0.0)
        with tc.tile_wait_until(0.005):
            nc.scalar.copy(ap(outa, 64, [[8, 120], [1, 8]]),       # site 0, lam > 0
                           ap(zq, 0, [[0, 120], [1, 8]]))
            nc.scalar.copy(ap(outa, 2, [[8, 8], [1, 4]]),          # site 0 row holes
                           ap(zq, 0, [[0, 8], [1, 4]]))
            nc.scalar.copy(ap(outd, 4, [[64, 16], [1, 60]]),       # site 19, rho > 0
                           ap(zq, 0, [[0, 16], [1, 60]]))
            for outt, qrel, nsites in ((outa, 1, 6), (outb, 0, 6), (outc, 0, 6)):
                nc.scalar.copy(
                    ap(outt, qrel * 1024 + 2, [[1024, nsites], [8, 128], [1, 4]]),
                    ap(zq, 0, [[0, nsites], [0, 128], [1, 4]]),
                )

        # ---- stage G: site 0 (m = 0 chain only) --------------------------
        for a1 in (0, 1):
            sdim = [[8, 2], [1, 4]]  # (a0, a2a3) scratch slice at a1*4
            A0 = lambda pl: ap(c01q, pl + a1 * 20, [[40, 2], [0, 4]])
            B0 = lambda pl: ap(c23q, pl + a1 * 80, [[0, 2], [20, 4]])
            nc.gpsimd.tensor_tensor(ap(s01, a1 * 4, sdim), A0(0), B0(0), MUL)
            nc.gpsimd.tensor_tensor(ap(s02, a1 * 4, sdim), A0(320), B0(640), MUL)
            nc.gpsimd.tensor_tensor(ap(s03, a1 * 4, sdim), A0(0), B0(640), MUL)
            nc.gpsimd.tensor_tensor(ap(s04, a1 * 4, sdim), A0(320), B0(0), MUL)
            o0 = [[32, 2], [8, 2], [6, 2]]
            sd2 = [[8, 2], [2, 2], [1, 2]]
            nc.gpsimd.tensor_sub(
                ap(outa, a1 * 16, o0), ap(s01, a1 * 4, sd2), ap(s02, a1 * 4, sd2)
            )
            nc.gpsimd.tensor_add(
                ap(outa, a1 * 16 + 1, o0), ap(s03, a1 * 4, sd2), ap(s04, a1 * 4, sd2)
            )

        def _emit_site19():
            # ---- stage H: site 19 (sum over a0,a1,a2; rho = 0) ---------------
            # scratch layout: a0*256 + a3*128 + lamA*32 + lamB*8 + a1*4? no:
            # (lamA,lamB,a1,a2) -> strides 16,4,2,1 within 64-block
            def p19_mult(dst, c01_pl, c23_pl):
                for a0 in (0, 1):
                    for a3 in (0, 1):
                        for a1 in (0, 1):
                            nc.vector.tensor_tensor(
                                ap(dst, a0 * 128 + a3 * 64 + a1 * 2, [[16, 4], [4, 4], [1, 2]]),
                                ap(c01q, c01_pl + (a0 * 2 + a1) * 20 + 19, [[80, 4], [0, 4], [0, 2]]),
                                ap(c23q, c23_pl + (a1 * 4 + a3) * 20 + 19, [[0, 4], [160, 4], [40, 2]]),
                                MUL,
                            )

            p19_mult(u19a, 0, 0)
            p19_mult(u19b, 320, 640)
            nc.vector.tensor_sub(pr19[:], u19a[:], u19b[:])
            p19_mult(u19a, 0, 640)
            p19_mult(u19b, 320, 0)
            nc.vector.tensor_add(pi19[:], u19a[:], u19b[:])
            # reduce a0 (stride 256), then a1 (stride 2), then a2 (stride 1)
            for src, d1, d2, dst in ((pr19, r1r, r2r, sr), (pi19, r1i, r2i, si)):
                nc.vector.tensor_add(d1[:], src[:, 0:128], src[:, 128:256])
                nc.vector.tensor_add(
                    ap(d2, 0, [[32, 2], [2, 16], [1, 2]]),
                    ap(d1, 0, [[64, 2], [4, 16], [1, 2]]),
                    ap(d1, 2, [[64, 2], [4, 16], [1, 2]]),
                )
                nc.vector.tensor_add(
                    ap(dst, 0, [[16, 2], [1, 16]]),
                    ap(d2, 0, [[32, 2], [2, 16]]),
                    ap(d2, 1, [[32, 2], [2, 16]]),
                )
            # scatter: out[19][lam, 0, i=a3] at lam*64 + a3*2 (+1 im)
            nc.scalar.copy(
                ap(outd, 0, [[2, 2], [64, 16]]), ap(sr, 0, [[16, 2], [1, 16]])
            )
            nc.scalar.copy(
                ap(outd, 1, [[2, 2], [64, 16]]), ap(si, 0, [[16, 2], [1, 16]])
            )
            nc.sync.dma_start(out_d[:, 19 * 1024 : 20 * 1024], outd[:])
        # ship the site 0 block early so outa's group DMA carries six sites
        nc.sync.dma_start(out_d[:, 0:1024], outa[:, 0:1024])
        import os
        PN = [int(x) for x in os.environ.get("KERN_POOL_NS", "2,2,2").split(",")]
        for gi, (outt, qb, qrel, nsq) in enumerate(
            ((outa, 1, 1, 6), (outb, 7, 0, 6), (outc, 13, 0, 6))
        ):
            pool_n = PN[gi]
            for a1 in (0, 1):
                for a2 in (0, 1):
                    for a3 in (0, 1):
                        trip = a1 * 4 + a2 * 2 + a3
                        scr = [[4 * nsq, 4], [nsq, 4], [1, nsq]]
                        if trip >= 8 - pool_n:
                            eng, w1, w2, w3, w4 = nc.gpsimd, tp1, tp2, tp3, tp4
                        else:
                            eng, w1, w2, w3, w4 = nc.vector, t1, t2, t3, t4
                        for a0 in (0, 1):
                            A = lambda pl: ap(
                                c01q, pl + (a0 * 2 + a1) * 20 + qb,
                                [[80, 4], [0, 4], [1, nsq]]
                            )
                            Bv = lambda pl: ap(
                                c23q,
                                pl + (a1 * 4 + a2 * 2 + a3) * 20 + qb,
                                [[0, 4], [160, 4], [1, nsq]],
                            )
                            h = (a0 * 2 + a1) * 128
                            eng.tensor_tensor(ap(w1, h, scr), A(0), Bv(0), MUL)
                            eng.tensor_tensor(ap(w2, h, scr), A(320), Bv(640), MUL)
                            eng.tensor_tensor(ap(w3, h, scr), A(0), Bv(640), MUL)
                            eng.tensor_tensor(ap(w4, h, scr), A(320), Bv(0), MUL)
                        ob = qrel * 1024 + a1 * 16 + a2 * 8 + a3 * 6
                        odims = [[1024, nsq], [64, 16], [32, 2]]
                        sdims = [[1, nsq], [nsq, 16], [256, 2]]
                        hh = a1 * 128
                        eng.tensor_sub(
                            ap(outt, ob, odims), ap(w1, hh, sdims), ap(w2, hh, sdims)
                        )
                        eng.tensor_add(
                            ap(outt, ob + 1, odims), ap(w3, hh, sdims), ap(w4, hh, sdims)
                        )
            if outt is outa:
                nc.sync.dma_start(out_d[:, 1024 : 7 * 1024], outa[:, 1024 : 7 * 1024])
            else:
                base = (qb - qrel) * 1024
                nc.sync.dma_start(out_d[:, base : base + nsq * 1024], outt[:])

        _emit_site19()

        # ---- stage F: wide expansion, interior sites ---------------------
        # out fp32 offset within site block: lamA*256 + lamB*64 + a0*32 + a1*16
        #                                    + a2*8 + a3*6 (+1 for im)

    nc.compile()
    return nc


def _get_nc():
    if "nc" not in _CACHE:
        _CACHE["nc"] = _build_nc()
    return _CACHE["nc"]


def kernel(theta, batch_size):
    from concourse.bass_utils import run_bass_kernel_spmd

    theta = np.ascontiguousarray(np.asarray(theta), dtype=np.float32)
    assert theta.shape == (B_TOTAL, P_COLS)
    nc = _get_nc()
    in_maps = [
        {"theta": theta[c * B : (c + 1) * B]} for c in range(N_CORES)
    ]
    res = run_bass_kernel_spmd(nc, in_maps, core_ids=list(range(N_CORES)))
    _CACHE["last_res"] = res
    full = np.concatenate([r["out"] for r in res.results], axis=0)  # [1024, 20480] f32
    return full.view(np.complex64).reshape(B_TOTAL, NQ, 16, 16, 2)



# revision 5
# speedup vs baseline: 3.2390x; 3.2390x over previous
"""Trainium2 Bass kernel for the batched MPS quantum-circuit forward pass.

Polar-form closed solution. Each gate is U_l = RZ(z)RY(y); the chained site
tensor entry is a product of 4 gate entries g_l(row_l, col_l) with
row_l = a_l ^ m_l, col_l = a_{l-1}.  Writing each entry as
(+-cos/sin(y_l/2)) * e^{+-i z_l/2} (phase sign = row bit), every output
element is  Mg * e^{i phi}  where Mg (signed magnitude) is a product of 4
real table entries and phi = sum_l (2*row_l - 1) * z_l/2 depends only on the
row bits t = (t0t1t2t3).

In t-coordinates (t_l = a_l ^ m_l) the value is independent of m3 and of the
rho-placement, so an interior site has only 8(m0m1m2) x 16(t) = 128 unique
complex values; the XOR placement rho = t ^ lam is a static permutation done
on the host.  Per site the device computes:
    Mg[m012, t] = M01x[m0, t0, t1] * M23[m1m2, t1t2t3]    (128 muls)
    im = Mg * sinT[t],  re = Mg * cosT[t]                 (2 x 128 muls)
with sinT/cosT built from a range-reduced phase table (phi/2pi - round, via
the f16 magic-constant trick) and one Sin activation each.

Output is compact f16: 18 interior sites x 256 (im|re planes) + site0 (16
complex, lam=0 only) + site19 (16 complex: sum over t0t1t2, index (m012,t3)).
The host expands to the full (1024,20,16,16,2) complex64 tensor (structural
zeros + XOR reindex); no host arithmetic beyond re+i*im.

Sharding: pure data parallelism, batch 1024 = 8 cores x 128 partitions.
All tensor instructions use <=3 free dims (hw TENSOR3D limit).
"""

import sys

sys.path.insert(0, "/opt/trn_rl_repo")

import numpy as np

B_TOTAL = 1024
N_CORES = 8
B = B_TOTAL // N_CORES
NQ = 20
P_COLS = 160
OUT_COLS = 4672  # 3*1536 interior + 32 site0 + 32 site19

_CACHE = {}


def _build_nc():
    import concourse.bass as bass
    import concourse.tile as tile
    from concourse import bacc, mybir

    f32 = mybir.dt.float32
    f16 = mybir.dt.float16
    ALU = mybir.AluOpType
    MUL = ALU.mult
    SIN = mybir.ActivationFunctionType.Sin
    ABS = mybir.ActivationFunctionType.Abs

    TWO_PI = float(2 * np.pi)
    INV_4PI = float(1.0 / (4 * np.pi))
    # Round-to-int magic for the f32 ALU datapath (storage dtype is f16 but
    # the DVE computes in f32, so the f32 magic is the one that rounds).
    MAGIC = 12582912.0  # 1.5 * 2^23

    nc = bacc.Bacc("TRN2", target_bir_lowering=False, debug=False)
    theta_d = nc.dram_tensor("theta", [B, P_COLS], f32, kind="ExternalInput").ap()
    out_d = nc.dram_tensor("out", [B, OUT_COLS], f16, kind="ExternalOutput").ap()

    from contextlib import ExitStack

    with tile.TileContext(nc) as tc, ExitStack() as ctx:
        pool = ctx.enter_context(tc.tile_pool(name="main", bufs=1))

        def tl(name, w, dt=f16):
            return pool.tile([B, w], dt, name=name)

        th = tl("th", 160, f32)
        ay = tl("ay", 80, f32)      # |y| (l*20+q)
        zs = tl("zs", 80)           # z/(4pi) f16 (l*20+q)
        pq = tl("pq", 160)          # q*8 + [P(t0t1):0..3 | Q(t2t3):4..7]
        phs = tl("phs", 320)        # q*16 + ts, ts = t0*8+t1*4+t2*2+t3
        kk = tl("kk", 320)          # round(phs)
        ww = tl("ww", 320)          # phs - round(phs) in [-.5,.5]
        nw = tl("nw", 320)
        aw = tl("aw", 320)
        sinT = tl("sinT", 320)      # sin(phi)  (plane 0 = im)
        cosT = tl("cosT", 320)      # cos(phi)
        gt = tl("gt", 320)          # (l*4 + row*2 + col)*20 + q
        m01x = tl("m01x", 640)      # q*32 + m0*16 + t0*8 + t1*4 + t2t3(dup)
        m23 = tl("m23", 640)        # q*32 + (m1*2+m2)*8 + t1*4 + t2*2 + t3
        mg = tl("mg", 2560)         # q*128 + m012*16 + ts
        outt = tl("outt", OUT_COLS)  # g*1536 + plane*768 + qg*128 + m012*16 + ts
        p19s = tl("p19s", 128)
        p19c = tl("p19c", 128)
        r1s = tl("r1s", 64)
        r2s = tl("r2s", 32)
        r1c = tl("r1c", 64)
        r2c = tl("r2c", 32)
        halfpi = tl("halfpi", 1, f32)
        warm = tl("warm", 1, f32)

        def ap(t, off, dims):
            w = t.shape[1]
            return bass.AP(tensor=t.tensor, offset=t.offset + off, ap=[[w, B]] + dims)

        # ---- warm the Sin activation table before theta arrives ----------
        nc.vector.memset(halfpi[:], float(np.pi / 2))
        nc.scalar.activation(warm[:], halfpi[:], SIN, scale=0.5)
        nc.sync.dma_start(th[:], theta_d)

        # ---- Act: gate tables -------------------------------------------
        yl = [[40, 4], [1, 20]]
        nc.scalar.activation(ap(ay, 0, [[20, 4], [1, 20]]), ap(th, 0, yl), ABS)
        nc.scalar.activation(
            ap(gt, 0, [[80, 4], [60, 2], [1, 20]]),
            ap(ay, 0, [[20, 4], [0, 2], [1, 20]]),
            SIN, bias=halfpi[:], scale=-0.5,
        )
        nc.scalar.activation(ap(gt, 40, [[80, 4], [1, 20]]), ap(th, 0, yl), SIN, scale=0.5)
        nc.scalar.activation(ap(gt, 20, [[80, 4], [1, 20]]), ap(th, 0, yl), SIN, scale=-0.5)

        # ---- DVE: phase chain -------------------------------------------
        nc.vector.tensor_scalar_mul(
            ap(zs, 0, [[20, 4], [1, 20]]), ap(th, 20, yl), INV_4PI
        )
        zin = lambda o: ap(zs, o, [[1, 20], [40, 2]])
        nc.vector.tensor_add(ap(pq, 3, [[8, 20], [4, 2]]), zin(0), zin(20))
        nc.vector.tensor_sub(ap(pq, 2, [[8, 20], [4, 2]]), zin(0), zin(20))
        nc.vector.tensor_scalar_mul(
            ap(pq, 0, [[8, 20], [4, 2], [1, 2]]),
            ap(pq, 3, [[8, 20], [4, 2], [-1, 2]]),
            -1.0,
        )
        # phs[q, ts] = P[t0t1] + Q[t2t3]; per-t0 instr, iter [t1, t2t3, q]
        for t0 in (0, 1):
            nc.vector.tensor_add(
                ap(phs, t0 * 8, [[4, 2], [1, 4], [16, 20]]),
                ap(pq, t0 * 2, [[1, 2], [0, 4], [8, 20]]),
                ap(pq, 4, [[0, 2], [1, 4], [8, 20]]),
            )
        nc.vector.tensor_scalar(kk[:], phs[:], MAGIC, MAGIC, ALU.add, ALU.subtract)
        nc.vector.tensor_sub(ww[:], phs[:], kk[:])
        nc.vector.tensor_scalar_mul(nw[:], ww[:], -1.0)
        nc.vector.tensor_tensor(aw[:], ww[:], nw[:], ALU.max)

        # ---- Act: phase trig --------------------------------------------
        nc.scalar.activation(sinT[:], ww[:], SIN, scale=TWO_PI)
        nc.scalar.activation(cosT[:], aw[:], SIN, bias=halfpi[:], scale=-TWO_PI)

        # ---- m23 on Pool(m1=0) + DVE(m1=1); m01x on DVE ------------------
        # m23[q,m12,t1t2t3] = g2(t2, t1^m1) * g3(t3, t2^m2); iter [t2,t3,q]
        def m23_build(eng, m1, m2, t1):
            eng.tensor_tensor(
                ap(m23, (m1 * 2 + m2) * 8 + t1 * 4, [[2, 2], [1, 2], [32, 20]]),
                ap(gt, 160 + ((t1 ^ m1) * 20), [[40, 2], [0, 2], [1, 20]]),
                ap(gt, 240 + m2 * 20, [[20 - m2 * 40, 2], [40, 2], [1, 20]]),
                MUL,
            )

        for m2 in (0, 1):
            for t1 in (0, 1):
                m23_build(nc.gpsimd, 0, m2, t1)
                m23_build(nc.vector, 1, m2, t1)

        # m01x[q, m0, t0, t1, dup t2t3] = g0(t0,0)*g1(t1, t0^m0); iter [q,t1,t2t3]
        for m0 in (0, 1):
            for t0 in (0, 1):
                nc.vector.tensor_tensor(
                    ap(m01x, m0 * 16 + t0 * 8, [[32, 20], [4, 2], [1, 4]]),
                    ap(gt, t0 * 40, [[1, 20], [0, 2], [0, 4]]),
                    ap(gt, 80 + ((t0 ^ m0) * 20), [[1, 20], [40, 2], [0, 4]]),
                    MUL,
                )

        # ---- mg: per (m0,t0), iter [q, m12, t123] ------------------------
        def mg_mul(q0, nq, m0, t0):
            nc.vector.tensor_tensor(
                ap(mg, q0 * 128 + m0 * 64 + t0 * 8, [[128, nq], [16, 4], [1, 8]]),
                ap(m01x, q0 * 32 + m0 * 16 + t0 * 8, [[32, nq], [0, 4], [1, 8]]),
                ap(m23, q0 * 32, [[32, nq], [8, 4], [1, 8]]),
                MUL,
            )

        def finals(g, q0, plane, T):
            nc.vector.tensor_tensor(
                ap(outt, g * 1536 + plane * 768, [[128, 6], [16, 8], [1, 16]]),
                ap(mg, q0 * 128, [[128, 6], [16, 8], [1, 16]]),
                ap(T, q0 * 16, [[16, 6], [0, 8], [1, 16]]),
                MUL,
            )

        # group A: q0..6 (site0 from q0 block)
        mg_mul(0, 7, 0, 0)
        mg_mul(0, 7, 0, 1)
        mg_mul(1, 6, 1, 0)
        mg_mul(1, 6, 1, 1)
        finals(0, 1, 0, sinT)
        nc.sync.dma_start(out_d[:, 0:768], outt[:, 0:768])
        finals(0, 1, 1, cosT)
        nc.sync.dma_start(out_d[:, 768:1536], outt[:, 768:1536])

        # site 0 on Pool: outt[4608 + plane*16 + ts] = mg[q0, m012=0] * T[q0]
        nc.gpsimd.tensor_tensor(
            ap(outt, 4608, [[1, 16]]), ap(mg, 0, [[1, 16]]), ap(sinT, 0, [[1, 16]]), MUL
        )
        nc.gpsimd.tensor_tensor(
            ap(outt, 4624, [[1, 16]]), ap(mg, 0, [[1, 16]]), ap(cosT, 0, [[1, 16]]), MUL
        )

        # groups B+C: q7..19 in one mg sweep
        mg_mul(7, 13, 0, 0)
        mg_mul(7, 13, 0, 1)
        mg_mul(7, 13, 1, 0)
        mg_mul(7, 13, 1, 1)
        finals(1, 7, 0, sinT)
        nc.sync.dma_start(out_d[:, 1536:2304], outt[:, 1536:2304])
        finals(1, 7, 1, cosT)
        nc.sync.dma_start(out_d[:, 2304:3072], outt[:, 2304:3072])
        finals(2, 13, 0, sinT)
        nc.sync.dma_start(out_d[:, 3072:3840], outt[:, 3072:3840])
        nc.vector.tensor_tensor(
            p19s[:], ap(mg, 19 * 128, [[16, 8], [1, 16]]),
            ap(sinT, 19 * 16, [[0, 8], [1, 16]]), MUL,
        )
        finals(2, 13, 1, cosT)
        nc.vector.tensor_tensor(
            p19c[:], ap(mg, 19 * 128, [[16, 8], [1, 16]]),
            ap(cosT, 19 * 16, [[0, 8], [1, 16]]), MUL,
        )
        nc.sync.dma_start(out_d[:, 3840:4608], outt[:, 3840:4608])
        # site 19 reduction over t0 (8), t2 (2), t3... strides in natural ts:
        # reduce t0 (stride 8), then t1 (stride 4), then t2 (stride 2), keep t3
        for p, r1, r2, oc in ((p19s, r1s, r2s, 4640), (p19c, r1c, r2c, 4656)):
            nc.gpsimd.tensor_add(
                ap(r1, 0, [[8, 8], [1, 8]]),
                ap(p, 0, [[16, 8], [1, 8]]), ap(p, 8, [[16, 8], [1, 8]]),
            )
            nc.gpsimd.tensor_add(
                ap(r2, 0, [[4, 8], [1, 4]]),
                ap(r1, 0, [[8, 8], [1, 4]]), ap(r1, 4, [[8, 8], [1, 4]]),
            )
            nc.gpsimd.tensor_add(
                ap(outt, oc, [[2, 8], [1, 2]]),
                ap(r2, 0, [[4, 8], [1, 2]]), ap(r2, 2, [[4, 8], [1, 2]]),
            )
        nc.sync.dma_start(out_d[:, 4608:4672], outt[:, 4608:4672])

    nc.compile()
    return nc


def _get_nc():
    if "nc" not in _CACHE:
        _CACHE["nc"] = _build_nc()
    return _CACHE["nc"]


# host-side static index maps (natural ts order: ts == t)
_LAM = np.arange(16)
_S = (_LAM[:, None] >> 1) * 16 + (_LAM[:, None] ^ _LAM[None, :])
_S19 = (_LAM[:, None] >> 1) * 2 + (np.arange(2)[None, :] ^ (_LAM[:, None] & 1))


def kernel(theta, batch_size):
    from concourse.bass_utils import run_bass_kernel_spmd

    theta = np.ascontiguousarray(np.asarray(theta), dtype=np.float32)
    assert theta.shape == (B_TOTAL, P_COLS)
    nc = _get_nc()
    in_maps = [{"theta": theta[c * B : (c + 1) * B]} for c in range(N_CORES)]
    res = run_bass_kernel_spmd(nc, in_maps, core_ids=list(range(N_CORES)))
    _CACHE["last_res"] = res
    buf = np.concatenate([r["out"] for r in res.results], axis=0).astype(np.float32)

    full = np.zeros((B_TOTAL, NQ, 16, 16, 2), np.complex64)
    ib = buf[:, :4608].reshape(B_TOTAL, 3, 2, 768)
    v = (ib[:, :, 1] + 1j * ib[:, :, 0]).reshape(B_TOTAL, 18, 128)  # [b,q,m012*16+ts]
    fi = v[:, :, _S]  # [b, 18, lam, rho]
    full[:, 1:19, :, 0::2, 0] = fi[..., 0::2]
    full[:, 1:19, :, 1::2, 1] = fi[..., 1::2]
    s0 = buf[:, 4608:4640].reshape(B_TOTAL, 2, 16)
    v0 = s0[:, 1] + 1j * s0[:, 0]
    full[:, 0, 0, 0::2, 0] = v0[:, 0::2]
    full[:, 0, 0, 1::2, 1] = v0[:, 1::2]
    s19 = buf[:, 4640:4672].reshape(B_TOTAL, 2, 16)
    v19 = s19[:, 1] + 1j * s19[:, 0]  # [b, m012*2+t3]
    full[:, 19, :, 0, 0] = v19[:, _S19[:, 0]]
    full[:, 19, :, 0, 1] = v19[:, _S19[:, 1]]
    return full


# revision 22
# speedup vs baseline: 3.4350x; 1.0605x over previous
"""Trainium2 Bass kernel for the batched MPS quantum-circuit forward pass.

Polar-form closed solution. Each gate is U_l = RZ(z)RY(y); the chained site
tensor entry is a product of 4 gate entries g_l(row_l, col_l) with
row_l = a_l ^ m_l, col_l = a_{l-1}.  Writing each entry as
(+-cos/sin(y_l/2)) * e^{+-i z_l/2} (phase sign = row bit), every output
element is  Mg * e^{i phi}  where Mg (signed magnitude) is a product of 4
real table entries and phi = sum_l (2*row_l - 1) * z_l/2 depends only on the
row bits t = (t0t1t2t3).

In t-coordinates (t_l = a_l ^ m_l) the value is independent of m3 and of the
rho-placement, so an interior site has only 8(m0m1m2) x 16(t) = 128 unique
complex values; the XOR placement rho = t ^ lam is a static permutation done
on the host.  Per site the device computes:
    Mg[m012, t] = M01x[m0, t0, t1] * M23[m1m2, t1t2t3]    (128 muls)
    im = Mg * sinT[t],  re = Mg * cosT[t]                 (2 x 128 muls)
with sinT/cosT built from a range-reduced phase table (phi/2pi - round, via
the f16 magic-constant trick) and one Sin activation each.

Output is compact f16: 18 interior sites x 256 (im|re planes) + site0 (16
complex, lam=0 only) + site19 (16 complex: sum over t0t1t2, index (m012,t3)).
The host expands to the full (1024,20,16,16,2) complex64 tensor (structural
zeros + XOR reindex); no host arithmetic beyond re+i*im.

Sharding: pure data parallelism, batch 1024 = 8 cores x 128 partitions.
All tensor instructions use <=3 free dims (hw TENSOR3D limit).
"""

import sys

sys.path.insert(0, "/opt/trn_rl_repo")

import numpy as np

B_TOTAL = 1024
N_CORES = 8
B = B_TOTAL // N_CORES
NQ = 20
P_COLS = 160
OUT_COLS = 4672  # 3*1536 interior + 32 site0 + 32 site19

_CACHE = {}


def _build_nc():
    import concourse.bass as bass
    import concourse.tile as tile
    from concourse import bacc, mybir

    f32 = mybir.dt.float32
    f16 = mybir.dt.float16
    ALU = mybir.AluOpType
    MUL = ALU.mult
    SIN = mybir.ActivationFunctionType.Sin
    ABS = mybir.ActivationFunctionType.Abs

    TWO_PI = float(2 * np.pi)
    INV_4PI = float(1.0 / (4 * np.pi))
    # Round-to-int magic for the f32 ALU datapath (storage dtype is f16 but
    # the DVE computes in f32, so the f32 magic is the one that rounds).
    MAGIC = 12582912.0  # 1.5 * 2^23

    nc = bacc.Bacc("TRN2", target_bir_lowering=False, debug=False)
    theta_d = nc.dram_tensor("theta", [B, P_COLS], f32, kind="ExternalInput").ap()
    out_d = nc.dram_tensor("out", [B, OUT_COLS], f16, kind="ExternalOutput").ap()

    from contextlib import ExitStack

    with tile.TileContext(nc) as tc, ExitStack() as ctx:
        pool = ctx.enter_context(tc.tile_pool(name="main", bufs=1))

        def tl(name, w, dt=f16):
            return pool.tile([B, w], dt, name=name)

        th = tl("th", 160, f32)
        ny = tl("ny", 80, f32)      # -y (l*20+q)
        ay = tl("ay", 80, f32)      # |y| (l*20+q)
        zs = tl("zs", 80)           # z/(4pi) f16 (l*20+q)
        pq = tl("pq", 160)          # q*8 + [P(t0t1):0..3 | Q(t2t3):4..7]
        phs = tl("phs", 320)        # q*16 + ts, ts = t0*8+t1*4+t2*2+t3
        kk = tl("kk", 320)          # round(phs)
        ww = tl("ww", 320)          # phs - round(phs) in [-.5,.5]
        nw = tl("nw", 320)
        aw = tl("aw", 320)
        sinT = tl("sinT", 320)      # sin(phi)  (plane 0 = im)
        cosT = tl("cosT", 320)      # cos(phi)
        gt = tl("gt", 320)          # (l*4 + row*2 + col)*20 + q
        m01x = tl("m01x", 640)      # q*32 + m0*16 + t0*8 + t1*4 + t2t3(dup)
        m23 = tl("m23", 640)        # q*32 + (m1*2+m2)*8 + t1*4 + t2*2 + t3
        mg = tl("mg", 2560)         # q*128 + m012*16 + ts
        outt = tl("outt", OUT_COLS)  # g*1536 + plane*768 + qg*128 + m012*16 + ts
        p19s = tl("p19s", 128)
        p19c = tl("p19c", 128)
        r1s = tl("r1s", 64)
        r2s = tl("r2s", 32)
        r1c = tl("r1c", 64)
        r2c = tl("r2c", 32)
        halfpi = tl("halfpi", 1, f32)
        warm = tl("warm", 1, f32)

        def ap(t, off, dims):
            w = t.shape[1]
            return bass.AP(tensor=t.tensor, offset=t.offset + off, ap=[[w, B]] + dims)

        # ---- warm the Sin activation table before theta arrives ----------
        nc.vector.memset(halfpi[:], float(np.pi / 2))
        nc.scalar.activation(warm[:], halfpi[:], SIN, scale=0.5)
        nc.sync.dma_start(th[:], theta_d)

        # ---- |y| on DVE (fills its idle start; shortens Act's chain) -----
        yl = [[40, 4], [1, 20]]
        al = [[20, 4], [1, 20]]
        nc.vector.tensor_scalar_mul(ap(ny, 0, al), ap(th, 0, yl), -1.0)
        nc.vector.tensor_tensor(ap(ay, 0, al), ap(th, 0, yl), ap(ny, 0, al), ALU.max)

        # ---- Act: gate tables -------------------------------------------
        nc.scalar.activation(
            ap(gt, 0, [[80, 4], [60, 2], [1, 20]]),
            ap(ay, 0, [[20, 4], [0, 2], [1, 20]]),
            SIN, bias=halfpi[:], scale=-0.5,
        )
        nc.scalar.activation(ap(gt, 40, [[80, 4], [1, 20]]), ap(th, 0, yl), SIN, scale=0.5)
        nc.scalar.activation(ap(gt, 20, [[80, 4], [1, 20]]), ap(th, 0, yl), SIN, scale=-0.5)

        # ---- DVE: phase chain -------------------------------------------
        nc.vector.tensor_scalar_mul(
            ap(zs, 0, [[20, 4], [1, 20]]), ap(th, 20, yl), INV_4PI
        )
        zin = lambda o: ap(zs, o, [[1, 20], [40, 2]])
        nc.vector.tensor_add(ap(pq, 3, [[8, 20], [4, 2]]), zin(0), zin(20))
        nc.vector.tensor_sub(ap(pq, 2, [[8, 20], [4, 2]]), zin(0), zin(20))
        nc.vector.tensor_scalar_mul(
            ap(pq, 0, [[8, 20], [4, 2], [1, 2]]),
            ap(pq, 3, [[8, 20], [4, 2], [-1, 2]]),
            -1.0,
        )
        # phs[q, ts] = P[t0t1] + Q[t2t3]; per-t0 instr, iter [t1, t2t3, q]
        for t0 in (0, 1):
            nc.vector.tensor_add(
                ap(phs, t0 * 8, [[4, 2], [1, 4], [16, 20]]),
                ap(pq, t0 * 2, [[1, 2], [0, 4], [8, 20]]),
                ap(pq, 4, [[0, 2], [1, 4], [8, 20]]),
            )
        nc.vector.tensor_scalar(kk[:], phs[:], MAGIC, MAGIC, ALU.add, ALU.subtract)
        nc.vector.tensor_sub(ww[:], phs[:], kk[:])

        # ---- Act: phase trig (abs on Act: it is idle waiting for ww) -----
        nc.scalar.activation(sinT[:], ww[:], SIN, scale=TWO_PI)
        nc.scalar.activation(aw[:], ww[:], ABS)
        nc.scalar.activation(cosT[:], aw[:], SIN, bias=halfpi[:], scale=-TWO_PI)

        # ---- m01x on DVE (early: Pool's mgBC half depends on it) ---------
        # m01x[q, m0, t0, t1, dup t2t3] = g0(t0,0)*g1(t1, t0^m0); iter [q,t1,t2t3]
        for m0 in (0, 1):
            for t0 in (0, 1):
                nc.vector.tensor_tensor(
                    ap(m01x, m0 * 16 + t0 * 8, [[32, 20], [4, 2], [1, 4]]),
                    ap(gt, t0 * 40, [[1, 20], [0, 2], [0, 4]]),
                    ap(gt, 80 + ((t0 ^ m0) * 20), [[1, 20], [40, 2], [0, 4]]),
                    MUL,
                )

        # ---- m23 on Pool (m1=0) + DVE (m1=1) -----------------------------
        # m23[q,m12,t1t2t3] = g2(t2, t1^m1) * g3(t3, t2^m2); iter [t2,t3,q]
        def m23_build(eng, m1, m2, t1):
            eng.tensor_tensor(
                ap(m23, (m1 * 2 + m2) * 8 + t1 * 4, [[2, 2], [1, 2], [32, 20]]),
                ap(gt, 160 + ((t1 ^ m1) * 20), [[40, 2], [0, 2], [1, 20]]),
                ap(gt, 240 + m2 * 20, [[20 - m2 * 40, 2], [40, 2], [1, 20]]),
                MUL,
            )

        for m1 in (0, 1):
            for m2 in (0, 1):
                for t1 in (0, 1):
                    eng = nc.vector if (m1, m2) == (1, 1) else nc.gpsimd
                    m23_build(eng, m1, m2, t1)

        # ---- mg: per (m0,t0), iter [q, m12, t123] ------------------------
        def mg_mul(q0, nq, m0, t0, eng=None):
            (eng or nc.vector).tensor_tensor(
                ap(mg, q0 * 128 + m0 * 64 + t0 * 8, [[128, nq], [16, 4], [1, 8]]),
                ap(m01x, q0 * 32 + m0 * 16 + t0 * 8, [[32, nq], [0, 4], [1, 8]]),
                ap(m23, q0 * 32, [[32, nq], [8, 4], [1, 8]]),
                MUL,
            )

        def finals(g, q0, plane, T):
            nc.vector.tensor_tensor(
                ap(outt, g * 1536 + plane * 768, [[128, 6], [16, 8], [1, 16]]),
                ap(mg, q0 * 128, [[128, 6], [16, 8], [1, 16]]),
                ap(T, q0 * 16, [[16, 6], [0, 8], [1, 16]]),
                MUL,
            )

        # group A: q0..6 (site0 from q0 block)
        mg_mul(0, 7, 0, 0)
        mg_mul(0, 7, 0, 1)
        mg_mul(1, 6, 1, 0)
        mg_mul(1, 6, 1, 1)
        with tc.high_priority():
            finals(0, 1, 0, sinT)
            finals(0, 1, 1, cosT)
            nc.sync.dma_start(out_d[:, 0:1536], outt[:, 0:1536])

        # groups B+C: q7..19; m0=1 half on Pool (its idle window)
        mg_mul(7, 13, 1, 0, nc.gpsimd)
        mg_mul(7, 13, 1, 1, nc.gpsimd)
        mg_mul(7, 13, 0, 0)
        mg_mul(7, 13, 0, 1)

        # site 0 on Pool: outt[4608 + plane*16 + ts] = mg[q0, m012=0] * T[q0]
        nc.gpsimd.tensor_tensor(
            ap(outt, 4608, [[1, 16]]), ap(mg, 0, [[1, 16]]), ap(sinT, 0, [[1, 16]]), MUL
        )
        nc.gpsimd.tensor_tensor(
            ap(outt, 4624, [[1, 16]]), ap(mg, 0, [[1, 16]]), ap(cosT, 0, [[1, 16]]), MUL
        )
        # site 19 (Pool, early so the merged C-re DMA is not tail-blocked)
        nc.gpsimd.tensor_tensor(
            p19s[:], ap(mg, 19 * 128, [[16, 8], [1, 16]]),
            ap(sinT, 19 * 16, [[0, 8], [1, 16]]), MUL,
        )
        nc.gpsimd.tensor_tensor(
            p19c[:], ap(mg, 19 * 128, [[16, 8], [1, 16]]),
            ap(cosT, 19 * 16, [[0, 8], [1, 16]]), MUL,
        )
        # reduce t0 (stride 8), then t1 (stride 4), then t2 (stride 2), keep t3
        for p, r1, r2, oc in ((p19s, r1s, r2s, 4640), (p19c, r1c, r2c, 4656)):
            nc.gpsimd.tensor_add(
                ap(r1, 0, [[8, 8], [1, 8]]),
                ap(p, 0, [[16, 8], [1, 8]]), ap(p, 8, [[16, 8], [1, 8]]),
            )
            nc.gpsimd.tensor_add(
                ap(r2, 0, [[4, 8], [1, 4]]),
                ap(r1, 0, [[8, 8], [1, 4]]), ap(r1, 4, [[8, 8], [1, 4]]),
            )
            nc.gpsimd.tensor_add(
                ap(outt, oc, [[2, 8], [1, 2]]),
                ap(r2, 0, [[4, 8], [1, 2]]), ap(r2, 2, [[4, 8], [1, 2]]),
            )
        with tc.high_priority():
            finals(1, 7, 0, sinT)
            nc.sync.dma_start(out_d[:, 1536:2304], outt[:, 1536:2304])
            finals(1, 7, 1, cosT)
            nc.sync.dma_start(out_d[:, 2304:3072], outt[:, 2304:3072])
            finals(2, 13, 0, sinT)
            nc.sync.dma_start(out_d[:, 3072:3840], outt[:, 3072:3840])
            finals(2, 13, 1, cosT)
            # C-re + site0 + site19 in one DMA
            nc.sync.dma_start(out_d[:, 3840:4672], outt[:, 3840:4672])

    nc.compile()
    return nc


def _get_nc():
    if "nc" not in _CACHE:
        _CACHE["nc"] = _build_nc()
    return _CACHE["nc"]


# host-side static index maps (natural ts order: ts == t)
_LAM = np.arange(16)
_S = (_LAM[:, None] >> 1) * 16 + (_LAM[:, None] ^ _LAM[None, :])
_S19 = (_LAM[:, None] >> 1) * 2 + (np.arange(2)[None, :] ^ (_LAM[:, None] & 1))


def kernel(theta, batch_size):
    from concourse.bass_utils import run_bass_kernel_spmd

    theta = np.ascontiguousarray(np.asarray(theta), dtype=np.float32)
    assert theta.shape == (B_TOTAL, P_COLS)
    nc = _get_nc()
    in_maps = [{"theta": theta[c * B : (c + 1) * B]} for c in range(N_CORES)]
    res = run_bass_kernel_spmd(nc, in_maps, core_ids=list(range(N_CORES)))
    _CACHE["last_res"] = res
    buf = np.concatenate([r["out"] for r in res.results], axis=0).astype(np.float32)

    full = np.zeros((B_TOTAL, NQ, 16, 16, 2), np.complex64)
    ib = buf[:, :4608].reshape(B_TOTAL, 3, 2, 768)
    v = (ib[:, :, 1] + 1j * ib[:, :, 0]).reshape(B_TOTAL, 18, 128)  # [b,q,m012*16+ts]
    fi = v[:, :, _S]  # [b, 18, lam, rho]
    full[:, 1:19, :, 0::2, 0] = fi[..., 0::2]
    full[:, 1:19, :, 1::2, 1] = fi[..., 1::2]
    s0 = buf[:, 4608:4640].reshape(B_TOTAL, 2, 16)
    v0 = s0[:, 1] + 1j * s0[:, 0]
    full[:, 0, 0, 0::2, 0] = v0[:, 0::2]
    full[:, 0, 0, 1::2, 1] = v0[:, 1::2]
    s19 = buf[:, 4640:4672].reshape(B_TOTAL, 2, 16)
    v19 = s19[:, 1] + 1j * s19[:, 0]  # [b, m012*2+t3]
    full[:, 19, :, 0, 0] = v19[:, _S19[:, 0]]
    full[:, 19, :, 0, 1] = v19[:, _S19[:, 1]]
    return full


# revision 28
# speedup vs baseline: 3.4989x; 1.0186x over previous
"""Trainium2 Bass kernel for the batched MPS quantum-circuit forward pass.

Polar-form closed solution. Each gate is U_l = RZ(z)RY(y); the chained site
tensor entry is a product of 4 gate entries g_l(row_l, col_l) with
row_l = a_l ^ m_l, col_l = a_{l-1}.  Writing each entry as
(+-cos/sin(y_l/2)) * e^{+-i z_l/2} (phase sign = row bit), every output
element is  Mg * e^{i phi}  where Mg (signed magnitude) is a product of 4
real table entries and phi = sum_l (2*row_l - 1) * z_l/2 depends only on the
row bits t = (t0t1t2t3).

In t-coordinates (t_l = a_l ^ m_l) the value is independent of m3 and of the
rho-placement, so an interior site has only 8(m0m1m2) x 16(t) = 128 unique
complex values; the XOR placement rho = t ^ lam is a static permutation done
on the host.  Per site the device computes:
    Mg[m012, t] = M01x[m0, t0, t1] * M23[m1m2, t1t2t3]    (128 muls)
    im = Mg * sinT[t],  re = Mg * cosT[t]                 (2 x 128 muls)
with sinT/cosT built from a range-reduced phase table (phi/2pi - round, via
the f16 magic-constant trick) and one Sin activation each.

Output is compact f16: 18 interior sites x 256 (im|re planes) + site0 (16
complex, lam=0 only) + site19 (16 complex: sum over t0t1t2, index (m012,t3)).
The host expands to the full (1024,20,16,16,2) complex64 tensor (structural
zeros + XOR reindex); no host arithmetic beyond re+i*im.

Sharding: pure data parallelism, batch 1024 = 8 cores x 128 partitions.
All tensor instructions use <=3 free dims (hw TENSOR3D limit).
"""

import sys

sys.path.insert(0, "/opt/trn_rl_repo")

import numpy as np

B_TOTAL = 1024
N_CORES = 8
B = B_TOTAL // N_CORES
NQ = 20
P_COLS = 160
OUT_COLS = 4672  # 3*1536 interior + 32 site0 + 32 site19

_CACHE = {}


def _build_nc():
    import concourse.bass as bass
    import concourse.tile as tile
    from concourse import bacc, mybir

    f32 = mybir.dt.float32
    f16 = mybir.dt.float16
    ALU = mybir.AluOpType
    MUL = ALU.mult
    SIN = mybir.ActivationFunctionType.Sin
    ABS = mybir.ActivationFunctionType.Abs

    TWO_PI = float(2 * np.pi)
    INV_4PI = float(1.0 / (4 * np.pi))
    # Round-to-int magic for the f32 ALU datapath (storage dtype is f16 but
    # the DVE computes in f32, so the f32 magic is the one that rounds).
    MAGIC = 12582912.0  # 1.5 * 2^23

    nc = bacc.Bacc("TRN2", target_bir_lowering=False, debug=False)
    theta_d = nc.dram_tensor("theta", [B, P_COLS], f32, kind="ExternalInput").ap()
    out_d = nc.dram_tensor("out", [B, OUT_COLS], f16, kind="ExternalOutput").ap()

    from contextlib import ExitStack

    with tile.TileContext(nc) as tc, ExitStack() as ctx:
        pool = ctx.enter_context(tc.tile_pool(name="main", bufs=1))

        def tl(name, w, dt=f16):
            return pool.tile([B, w], dt, name=name)

        th = tl("th", 160, f32)
        zs = tl("zs", 80)           # z/(4pi) f16 (l*20+q)
        pq = tl("pq", 160)          # q*8 + [P(t0t1):0..3 | Q(t2t3):4..7]
        phs = tl("phs", 320)        # q*16 + ts, ts = t0*8+t1*4+t2*2+t3
        kk = tl("kk", 320)          # round(phs)
        ww = tl("ww", 320)          # phs - round(phs) in [-.5,.5]
        nw = tl("nw", 320)
        aw = tl("aw", 320)
        sinT = tl("sinT", 320)      # sin(phi)  (plane 0 = im)
        cosT = tl("cosT", 320)      # cos(phi)
        gt = tl("gt", 320)          # (l*4 + row*2 + col)*20 + q
        m01x = tl("m01x", 640)      # q*32 + m0*16 + t0*8 + t1*4 + t2t3(dup)
        m23 = tl("m23", 640)        # q*32 + (m1*2+m2)*8 + t1*4 + t2*2 + t3
        mg = tl("mg", 2560)         # q*128 + m012*16 + ts
        outt = tl("outt", OUT_COLS)  # g*1536 + plane*768 + qg*128 + m012*16 + ts
        p19s = tl("p19s", 128)
        p19c = tl("p19c", 128)
        r1s = tl("r1s", 64)
        r2s = tl("r2s", 32)
        r1c = tl("r1c", 64)
        r2c = tl("r2c", 32)
        halfpi = tl("halfpi", 1, f32)
        warm = tl("warm", 1, f32)

        def ap(t, off, dims):
            w = t.shape[1]
            return bass.AP(tensor=t.tensor, offset=t.offset + off, ap=[[w, B]] + dims)

        # ---- warm the Sin activation table before theta arrives ----------
        nc.vector.memset(halfpi[:], float(np.pi / 2))
        nc.scalar.activation(warm[:], halfpi[:], SIN, scale=0.5)
        nc.sync.dma_start(th[:], theta_d)

        # ---- Act: gate tables -------------------------------------------
        # cos(y/2) = Sin(pi/2 - y/2) directly; for the rare y < -pi the Sin
        # table arg exceeds pi where its error is still only ~1e-3 (measured).
        yl = [[40, 4], [1, 20]]
        nc.scalar.activation(
            ap(gt, 0, [[80, 4], [60, 2], [1, 20]]),
            ap(th, 0, [[40, 4], [0, 2], [1, 20]]),
            SIN, bias=halfpi[:], scale=-0.5,
        )
        nc.scalar.activation(ap(gt, 40, [[80, 4], [1, 20]]), ap(th, 0, yl), SIN, scale=0.5)
        nc.scalar.activation(ap(gt, 20, [[80, 4], [1, 20]]), ap(th, 0, yl), SIN, scale=-0.5)

        # ---- DVE: phase chain -------------------------------------------
        nc.vector.tensor_scalar_mul(
            ap(zs, 0, [[20, 4], [1, 20]]), ap(th, 20, yl), INV_4PI
        )
        zin = lambda o: ap(zs, o, [[1, 20], [40, 2]])
        nc.vector.tensor_add(ap(pq, 3, [[8, 20], [4, 2]]), zin(0), zin(20))
        nc.vector.tensor_sub(ap(pq, 2, [[8, 20], [4, 2]]), zin(0), zin(20))
        nc.vector.tensor_scalar_mul(
            ap(pq, 0, [[8, 20], [4, 2], [1, 2]]),
            ap(pq, 3, [[8, 20], [4, 2], [-1, 2]]),
            -1.0,
        )
        # phs[q, ts] = P[t0t1] + Q[t2t3]; per-t0 instr, iter [t1, t2t3, q]
        for t0 in (0, 1):
            nc.vector.tensor_add(
                ap(phs, t0 * 8, [[4, 2], [1, 4], [16, 20]]),
                ap(pq, t0 * 2, [[1, 2], [0, 4], [8, 20]]),
                ap(pq, 4, [[0, 2], [1, 4], [8, 20]]),
            )
        nc.vector.tensor_scalar(kk[:], phs[:], MAGIC, MAGIC, ALU.add, ALU.subtract)
        nc.vector.tensor_sub(ww[:], phs[:], kk[:])

        # ---- Act: phase trig (abs on Act: it is idle waiting for ww) -----
        nc.scalar.activation(sinT[:], ww[:], SIN, scale=TWO_PI)
        nc.scalar.activation(aw[:], ww[:], ABS)
        nc.scalar.activation(cosT[:], aw[:], SIN, bias=halfpi[:], scale=-TWO_PI)

        # ---- m01x on DVE (early: Pool's mgBC half depends on it) ---------
        # m01x[q, m0, t0, t1, dup t2t3] = g0(t0,0)*g1(t1, t0^m0); iter [q,t1,t2t3]
        for m0 in (0, 1):
            for t0 in (0, 1):
                nc.vector.tensor_tensor(
                    ap(m01x, m0 * 16 + t0 * 8, [[32, 20], [4, 2], [1, 4]]),
                    ap(gt, t0 * 40, [[1, 20], [0, 2], [0, 4]]),
                    ap(gt, 80 + ((t0 ^ m0) * 20), [[1, 20], [40, 2], [0, 4]]),
                    MUL,
                )

        # ---- m23 on Pool (m1=0) + DVE (m1=1) -----------------------------
        # m23[q,m12,t1t2t3] = g2(t2, t1^m1) * g3(t3, t2^m2); iter [t2,t3,q]
        def m23_build(eng, m1, m2, t1):
            eng.tensor_tensor(
                ap(m23, (m1 * 2 + m2) * 8 + t1 * 4, [[2, 2], [1, 2], [32, 20]]),
                ap(gt, 160 + ((t1 ^ m1) * 20), [[40, 2], [0, 2], [1, 20]]),
                ap(gt, 240 + m2 * 20, [[20 - m2 * 40, 2], [40, 2], [1, 20]]),
                MUL,
            )

        for m1 in (0, 1):
            for m2 in (0, 1):
                for t1 in (0, 1):
                    eng = nc.vector if (m1, m2) == (1, 1) else nc.gpsimd
                    m23_build(eng, m1, m2, t1)

        # ---- mg: per (m0,t0), iter [q, m12, t123] ------------------------
        def mg_mul(q0, nq, m0, t0, eng=None):
            (eng or nc.vector).tensor_tensor(
                ap(mg, q0 * 128 + m0 * 64 + t0 * 8, [[128, nq], [16, 4], [1, 8]]),
                ap(m01x, q0 * 32 + m0 * 16 + t0 * 8, [[32, nq], [0, 4], [1, 8]]),
                ap(m23, q0 * 32, [[32, nq], [8, 4], [1, 8]]),
                MUL,
            )

        def finals(base, q0, nq, plane, T):
            nc.vector.tensor_tensor(
                ap(outt, base + plane * nq * 128, [[128, nq], [16, 8], [1, 16]]),
                ap(mg, q0 * 128, [[128, nq], [16, 8], [1, 16]]),
                ap(T, q0 * 16, [[16, nq], [0, 8], [1, 16]]),
                MUL,
            )

        # group A: q1..6 (site0 from q0 block)
        mg_mul(0, 7, 0, 0)
        mg_mul(0, 7, 0, 1)
        mg_mul(1, 6, 1, 0)
        mg_mul(1, 6, 1, 1)
        with tc.high_priority():
            finals(0, 1, 6, 0, sinT)
            nc.sync.dma_start(out_d[:, 0:768], outt[:, 0:768])
            finals(0, 1, 6, 1, cosT)
            nc.sync.dma_start(out_d[:, 768:1536], outt[:, 768:1536])

        # groups B+C: q7..19; m0=1 half on Pool (its idle window)
        mg_mul(7, 13, 1, 0, nc.gpsimd)
        mg_mul(7, 13, 1, 1, nc.gpsimd)
        mg_mul(7, 13, 0, 0)
        mg_mul(7, 13, 0, 1)

        # site 0 on Pool: outt[4608 + plane*16 + ts] = mg[q0, m012=0] * T[q0]
        nc.gpsimd.tensor_tensor(
            ap(outt, 4608, [[1, 16]]), ap(mg, 0, [[1, 16]]), ap(sinT, 0, [[1, 16]]), MUL
        )
        nc.gpsimd.tensor_tensor(
            ap(outt, 4624, [[1, 16]]), ap(mg, 0, [[1, 16]]), ap(cosT, 0, [[1, 16]]), MUL
        )
        # site 19 (Pool, early so the merged C-re DMA is not tail-blocked)
        nc.gpsimd.tensor_tensor(
            p19s[:], ap(mg, 19 * 128, [[16, 8], [1, 16]]),
            ap(sinT, 19 * 16, [[0, 8], [1, 16]]), MUL,
        )
        nc.gpsimd.tensor_tensor(
            p19c[:], ap(mg, 19 * 128, [[16, 8], [1, 16]]),
            ap(cosT, 19 * 16, [[0, 8], [1, 16]]), MUL,
        )
        # reduce t0 (stride 8), then t1 (stride 4), then t2 (stride 2), keep t3
        for p, r1, r2, oc in ((p19s, r1s, r2s, 4640), (p19c, r1c, r2c, 4656)):
            nc.gpsimd.tensor_add(
                ap(r1, 0, [[8, 8], [1, 8]]),
                ap(p, 0, [[16, 8], [1, 8]]), ap(p, 8, [[16, 8], [1, 8]]),
            )
            nc.gpsimd.tensor_add(
                ap(r2, 0, [[4, 8], [1, 4]]),
                ap(r1, 0, [[8, 8], [1, 4]]), ap(r1, 4, [[8, 8], [1, 4]]),
            )
            nc.gpsimd.tensor_add(
                ap(outt, oc, [[2, 8], [1, 2]]),
                ap(r2, 0, [[4, 8], [1, 2]]), ap(r2, 2, [[4, 8], [1, 2]]),
            )
        with tc.high_priority():
            finals(1536, 7, 8, 0, sinT)
            nc.sync.dma_start(out_d[:, 1536:2560], outt[:, 1536:2560])
            finals(1536, 7, 8, 1, cosT)
            nc.sync.dma_start(out_d[:, 2560:3584], outt[:, 2560:3584])
            finals(3584, 15, 4, 0, sinT)
            finals(3584, 15, 4, 1, cosT)
            # whole group C + site0 + site19 in one DMA
            nc.sync.dma_start(out_d[:, 3584:4672], outt[:, 3584:4672])

    nc.compile()
    return nc


def _get_nc():
    if "nc" not in _CACHE:
        _CACHE["nc"] = _build_nc()
    return _CACHE["nc"]


# host-side static index maps (natural ts order: ts == t)
_LAM = np.arange(16)
_S = (_LAM[:, None] >> 1) * 16 + (_LAM[:, None] ^ _LAM[None, :])
_S19 = (_LAM[:, None] >> 1) * 2 + (np.arange(2)[None, :] ^ (_LAM[:, None] & 1))


def kernel(theta, batch_size):
    from concourse.bass_utils import run_bass_kernel_spmd

    theta = np.ascontiguousarray(np.asarray(theta), dtype=np.float32)
    assert theta.shape == (B_TOTAL, P_COLS)
    nc = _get_nc()
    in_maps = [{"theta": theta[c * B : (c + 1) * B]} for c in range(N_CORES)]
    res = run_bass_kernel_spmd(nc, in_maps, core_ids=list(range(N_CORES)))
    _CACHE["last_res"] = res
    buf = np.concatenate([r["out"] for r in res.results], axis=0).astype(np.float32)

    full = np.zeros((B_TOTAL, NQ, 16, 16, 2), np.complex64)
    vs = []
    for base, nq in ((0, 6), (1536, 8), (3584, 4)):
        g = buf[:, base : base + nq * 256].reshape(B_TOTAL, 2, nq, 128)
        vs.append(g[:, 1] + 1j * g[:, 0])
    v = np.concatenate(vs, axis=1)  # [b, 18, m012*16+ts]
    fi = v[:, :, _S]  # [b, 18, lam, rho]
    full[:, 1:19, :, 0::2, 0] = fi[..., 0::2]
    full[:, 1:19, :, 1::2, 1] = fi[..., 1::2]
    s0 = buf[:, 4608:4640].reshape(B_TOTAL, 2, 16)
    v0 = s0[:, 1] + 1j * s0[:, 0]
    full[:, 0, 0, 0::2, 0] = v0[:, 0::2]
    full[:, 0, 0, 1::2, 1] = v0[:, 1::2]
    s19 = buf[:, 4640:4672].reshape(B_TOTAL, 2, 16)
    v19 = s19[:, 1] + 1j * s19[:, 0]  # [b, m012*2+t3]
    full[:, 19, :, 0, 0] = v19[:, _S19[:, 0]]
    full[:, 19, :, 0, 1] = v19[:, _S19[:, 1]]
    return full


# revision 32
# speedup vs baseline: 3.5318x; 1.0094x over previous
"""Trainium2 Bass kernel for the batched MPS quantum-circuit forward pass.

Polar-form closed solution. Each gate is U_l = RZ(z)RY(y); the chained site
tensor entry is a product of 4 gate entries g_l(row_l, col_l) with
row_l = a_l ^ m_l, col_l = a_{l-1}.  Writing each entry as
(+-cos/sin(y_l/2)) * e^{+-i z_l/2} (phase sign = row bit), every output
element is  Mg * e^{i phi}  where Mg (signed magnitude) is a product of 4
real table entries and phi = sum_l (2*row_l - 1) * z_l/2 depends only on the
row bits t = (t0t1t2t3).

In t-coordinates (t_l = a_l ^ m_l) the value is independent of m3 and of the
rho-placement, so an interior site has only 8(m0m1m2) x 16(t) = 128 unique
complex values; the XOR placement rho = t ^ lam is a static permutation done
on the host.  Per site the device computes:
    Mg[m012, t] = M01x[m0, t0, t1] * M23[m1m2, t1t2t3]    (128 muls)
    im = Mg * sinT[t],  re = Mg * cosT[t]                 (2 x 128 muls)
with sinT/cosT built from a range-reduced phase table (phi/2pi - round, via
the f16 magic-constant trick) and one Sin activation each.

Output is compact f16: 18 interior sites x 256 (im|re planes) + site0 (16
complex, lam=0 only) + site19 (16 complex: sum over t0t1t2, index (m012,t3)).
The host expands to the full (1024,20,16,16,2) complex64 tensor (structural
zeros + XOR reindex); no host arithmetic beyond re+i*im.

Sharding: pure data parallelism, batch 1024 = 8 cores x 128 partitions.
All tensor instructions use <=3 free dims (hw TENSOR3D limit).
"""

import sys

sys.path.insert(0, "/opt/trn_rl_repo")

import numpy as np

B_TOTAL = 1024
N_CORES = 8
B = B_TOTAL // N_CORES
NQ = 20
P_COLS = 160
OUT_COLS = 4672  # 3*1536 interior + 32 site0 + 32 site19

_CACHE = {}


def _build_nc():
    import concourse.bass as bass
    import concourse.tile as tile
    from concourse import bacc, mybir

    f32 = mybir.dt.float32
    f16 = mybir.dt.float16
    ALU = mybir.AluOpType
    MUL = ALU.mult
    SIN = mybir.ActivationFunctionType.Sin
    ABS = mybir.ActivationFunctionType.Abs

    TWO_PI = float(2 * np.pi)
    INV_4PI = float(1.0 / (4 * np.pi))
    # Round-to-int magic for the f32 ALU datapath (storage dtype is f16 but
    # the DVE computes in f32, so the f32 magic is the one that rounds).
    MAGIC = 12582912.0  # 1.5 * 2^23

    nc = bacc.Bacc("TRN2", target_bir_lowering=False, debug=False)
    theta_d = nc.dram_tensor("theta", [B, P_COLS], f32, kind="ExternalInput").ap()
    out_d = nc.dram_tensor("out", [B, OUT_COLS], f16, kind="ExternalOutput").ap()

    from contextlib import ExitStack

    with tile.TileContext(nc) as tc, ExitStack() as ctx:
        pool = ctx.enter_context(tc.tile_pool(name="main", bufs=1))

        def tl(name, w, dt=f16):
            return pool.tile([B, w], dt, name=name)

        th = tl("th", 160, f32)
        zs = tl("zs", 80)           # z/(4pi) f16 (l*20+q)
        pq = tl("pq", 160)          # q*8 + [P(t0t1):0..3 | Q(t2t3):4..7]
        phs = tl("phs", 320)        # q*16 + ts, ts = t0*8+t1*4+t2*2+t3
        kk = tl("kk", 320)          # round(phs)
        ww = tl("ww", 320)          # phs - round(phs) in [-.5,.5]
        nw = tl("nw", 320)
        aw = tl("aw", 320)
        sinT = tl("sinT", 320)      # sin(phi)  (plane 0 = im)
        cosT = tl("cosT", 320)      # cos(phi)
        gt = tl("gt", 320)          # (l*4 + row*2 + col)*20 + q
        m01x = tl("m01x", 640)      # q*32 + m0*16 + t0*8 + t1*4 + t2t3(dup)
        m23 = tl("m23", 640)        # q*32 + (m1*2+m2)*8 + t1*4 + t2*2 + t3
        mg = tl("mg", 2560)         # q*128 + m012*16 + ts
        outt = tl("outt", OUT_COLS)  # g*1536 + plane*768 + qg*128 + m012*16 + ts
        p19s = tl("p19s", 128)
        p19c = tl("p19c", 128)
        r1s = tl("r1s", 64)
        r2s = tl("r2s", 32)
        r1c = tl("r1c", 64)
        r2c = tl("r2c", 32)
        halfpi = tl("halfpi", 1, f32)
        warm = tl("warm", 1, f32)

        def ap(t, off, dims):
            w = t.shape[1]
            return bass.AP(tensor=t.tensor, offset=t.offset + off, ap=[[w, B]] + dims)

        # ---- warm the Sin activation table before theta arrives ----------
        nc.vector.memset(halfpi[:], float(np.pi / 2))
        nc.scalar.activation(warm[:], halfpi[:], SIN, scale=0.5)
        nc.sync.dma_start(th[:], theta_d)

        # ---- Act: gate tables -------------------------------------------
        # cos(y/2) = Sin(pi/2 - y/2) directly; for the rare y < -pi the Sin
        # table arg exceeds pi where its error is still only ~1e-3 (measured).
        yl = [[40, 4], [1, 20]]
        nc.scalar.activation(
            ap(gt, 0, [[80, 4], [60, 2], [1, 20]]),
            ap(th, 0, [[40, 4], [0, 2], [1, 20]]),
            SIN, bias=halfpi[:], scale=-0.5,
        )
        nc.scalar.activation(ap(gt, 40, [[80, 4], [1, 20]]), ap(th, 0, yl), SIN, scale=0.5)
        nc.scalar.activation(ap(gt, 20, [[80, 4], [1, 20]]), ap(th, 0, yl), SIN, scale=-0.5)

        # ---- DVE: phase chain -------------------------------------------
        nc.vector.tensor_scalar_mul(
            ap(zs, 0, [[20, 4], [1, 20]]), ap(th, 20, yl), INV_4PI
        )
        zin = lambda o: ap(zs, o, [[1, 20], [40, 2]])
        nc.vector.tensor_add(ap(pq, 3, [[8, 20], [4, 2]]), zin(0), zin(20))
        nc.vector.tensor_sub(ap(pq, 2, [[8, 20], [4, 2]]), zin(0), zin(20))
        nc.vector.tensor_scalar_mul(
            ap(pq, 0, [[8, 20], [4, 2], [1, 2]]),
            ap(pq, 3, [[8, 20], [4, 2], [-1, 2]]),
            -1.0,
        )
        # phs[q, ts] = P[t0t1] + Q[t2t3]; per-t0 instr, iter [t1, t2t3, q]
        for t0 in (0, 1):
            nc.vector.tensor_add(
                ap(phs, t0 * 8, [[4, 2], [1, 4], [16, 20]]),
                ap(pq, t0 * 2, [[1, 2], [0, 4], [8, 20]]),
                ap(pq, 4, [[0, 2], [1, 4], [8, 20]]),
            )
        nc.vector.tensor_scalar(kk[:], phs[:], MAGIC, MAGIC, ALU.add, ALU.subtract)
        nc.vector.tensor_sub(ww[:], phs[:], kk[:])

        # ---- Act: phase trig (abs on Act: it is idle waiting for ww) -----
        nc.scalar.activation(sinT[:], ww[:], SIN, scale=TWO_PI)
        nc.scalar.activation(aw[:], ww[:], ABS)
        nc.scalar.activation(cosT[:], aw[:], SIN, bias=halfpi[:], scale=-TWO_PI)

        # ---- m01x on DVE (early: Pool's mgBC half depends on it) ---------
        # m01x[q, m0, t0, t1, dup t2t3] = g0(t0,0)*g1(t1, t0^m0); iter [q,t1,t2t3]
        with tc.high_priority():
            for m0 in (0, 1):
                for t0 in (0, 1):
                    nc.vector.tensor_tensor(
                        ap(m01x, m0 * 16 + t0 * 8, [[32, 20], [4, 2], [1, 4]]),
                        ap(gt, t0 * 40, [[1, 20], [0, 2], [0, 4]]),
                        ap(gt, 80 + ((t0 ^ m0) * 20), [[1, 20], [40, 2], [0, 4]]),
                        MUL,
                    )

        # ---- m23 on Pool (m1=0) + DVE (m1=1) -----------------------------
        # m23[q,m12,t1t2t3] = g2(t2, t1^m1) * g3(t3, t2^m2); iter [t2,t3,q]
        def m23_build(eng, m1, m2, t1):
            eng.tensor_tensor(
                ap(m23, (m1 * 2 + m2) * 8 + t1 * 4, [[2, 2], [1, 2], [32, 20]]),
                ap(gt, 160 + ((t1 ^ m1) * 20), [[40, 2], [0, 2], [1, 20]]),
                ap(gt, 240 + m2 * 20, [[20 - m2 * 40, 2], [40, 2], [1, 20]]),
                MUL,
            )

        for m1 in (0, 1):
            for m2 in (0, 1):
                for t1 in (0, 1):
                    eng = nc.vector if (m1, m2) == (1, 1) else nc.gpsimd
                    m23_build(eng, m1, m2, t1)

        # ---- mg: per (m0,t0), iter [q, m12, t123] ------------------------
        def mg_mul(q0, nq, m0, t0, eng=None):
            (eng or nc.vector).tensor_tensor(
                ap(mg, q0 * 128 + m0 * 64 + t0 * 8, [[128, nq], [16, 4], [1, 8]]),
                ap(m01x, q0 * 32 + m0 * 16 + t0 * 8, [[32, nq], [0, 4], [1, 8]]),
                ap(m23, q0 * 32, [[32, nq], [8, 4], [1, 8]]),
                MUL,
            )

        def finals(base, q0, nq, plane, T):
            nc.vector.tensor_tensor(
                ap(outt, base + plane * nq * 128, [[128, nq], [16, 8], [1, 16]]),
                ap(mg, q0 * 128, [[128, nq], [16, 8], [1, 16]]),
                ap(T, q0 * 16, [[16, nq], [0, 8], [1, 16]]),
                MUL,
            )

        # group A: q1..6 (site0 from q0 block)
        mg_mul(0, 7, 0, 0)
        mg_mul(0, 7, 0, 1)
        mg_mul(1, 6, 1, 0)
        mg_mul(1, 6, 1, 1)
        with tc.high_priority():
            finals(0, 1, 6, 0, sinT)
            nc.sync.dma_start(out_d[:, 0:768], outt[:, 0:768])
            finals(0, 1, 6, 1, cosT)
            nc.sync.dma_start(out_d[:, 768:1536], outt[:, 768:1536])

        # groups B+C: q7..19; m0=1 half on Pool (its idle window)
        mg_mul(7, 13, 1, 0, nc.gpsimd)
        mg_mul(7, 13, 1, 1, nc.gpsimd)
        mg_mul(7, 13, 0, 0)
        mg_mul(7, 13, 0, 1)

        # site 0 on Pool: outt[4608 + plane*16 + ts] = mg[q0, m012=0] * T[q0]
        nc.gpsimd.tensor_tensor(
            ap(outt, 4608, [[1, 16]]), ap(mg, 0, [[1, 16]]), ap(sinT, 0, [[1, 16]]), MUL
        )
        nc.gpsimd.tensor_tensor(
            ap(outt, 4624, [[1, 16]]), ap(mg, 0, [[1, 16]]), ap(cosT, 0, [[1, 16]]), MUL
        )
        # site 19 (Pool, early so the merged C-re DMA is not tail-blocked)
        nc.gpsimd.tensor_tensor(
            p19s[:], ap(mg, 19 * 128, [[16, 8], [1, 16]]),
            ap(sinT, 19 * 16, [[0, 8], [1, 16]]), MUL,
        )
        nc.gpsimd.tensor_tensor(
            p19c[:], ap(mg, 19 * 128, [[16, 8], [1, 16]]),
            ap(cosT, 19 * 16, [[0, 8], [1, 16]]), MUL,
        )
        # reduce t0 (stride 8), then t1 (stride 4), then t2 (stride 2), keep t3
        for p, r1, r2, oc in ((p19s, r1s, r2s, 4640), (p19c, r1c, r2c, 4656)):
            nc.gpsimd.tensor_add(
                ap(r1, 0, [[8, 8], [1, 8]]),
                ap(p, 0, [[16, 8], [1, 8]]), ap(p, 8, [[16, 8], [1, 8]]),
            )
            nc.gpsimd.tensor_add(
                ap(r2, 0, [[4, 8], [1, 4]]),
                ap(r1, 0, [[8, 8], [1, 4]]), ap(r1, 4, [[8, 8], [1, 4]]),
            )
            nc.gpsimd.tensor_add(
                ap(outt, oc, [[2, 8], [1, 2]]),
                ap(r2, 0, [[4, 8], [1, 2]]), ap(r2, 2, [[4, 8], [1, 2]]),
            )
        with tc.high_priority():
            finals(1536, 7, 8, 0, sinT)
            nc.sync.dma_start(out_d[:, 1536:2560], outt[:, 1536:2560])
            finals(1536, 7, 8, 1, cosT)
            nc.sync.dma_start(out_d[:, 2560:3584], outt[:, 2560:3584])
            finals(3584, 15, 4, 0, sinT)
            finals(3584, 15, 4, 1, cosT)
            # whole group C + site0 + site19 in one DMA
            nc.sync.dma_start(out_d[:, 3584:4672], outt[:, 3584:4672])

    nc.compile()
    return nc


def _get_nc():
    if "nc" not in _CACHE:
        _CACHE["nc"] = _build_nc()
    return _CACHE["nc"]


# host-side static index maps (natural ts order: ts == t)
_LAM = np.arange(16)
_S = (_LAM[:, None] >> 1) * 16 + (_LAM[:, None] ^ _LAM[None, :])
_S19 = (_LAM[:, None] >> 1) * 2 + (np.arange(2)[None, :] ^ (_LAM[:, None] & 1))


def kernel(theta, batch_size):
    from concourse.bass_utils import run_bass_kernel_spmd

    theta = np.ascontiguousarray(np.asarray(theta), dtype=np.float32)
    assert theta.shape == (B_TOTAL, P_COLS)
    nc = _get_nc()
    in_maps = [{"theta": theta[c * B : (c + 1) * B]} for c in range(N_CORES)]
    res = run_bass_kernel_spmd(nc, in_maps, core_ids=list(range(N_CORES)))
    _CACHE["last_res"] = res
    buf = np.concatenate([r["out"] for r in res.results], axis=0).astype(np.float32)

    full = np.zeros((B_TOTAL, NQ, 16, 16, 2), np.complex64)
    vs = []
    for base, nq in ((0, 6), (1536, 8), (3584, 4)):
        g = buf[:, base : base + nq * 256].reshape(B_TOTAL, 2, nq, 128)
        vs.append(g[:, 1] + 1j * g[:, 0])
    v = np.concatenate(vs, axis=1)  # [b, 18, m012*16+ts]
    fi = v[:, :, _S]  # [b, 18, lam, rho]
    full[:, 1:19, :, 0::2, 0] = fi[..., 0::2]
    full[:, 1:19, :, 1::2, 1] = fi[..., 1::2]
    s0 = buf[:, 4608:4640].reshape(B_TOTAL, 2, 16)
    v0 = s0[:, 1] + 1j * s0[:, 0]
    full[:, 0, 0, 0::2, 0] = v0[:, 0::2]
    full[:, 0, 0, 1::2, 1] = v0[:, 1::2]
    s19 = buf[:, 4640:4672].reshape(B_TOTAL, 2, 16)
    v19 = s19[:, 1] + 1j * s19[:, 0]  # [b, m012*2+t3]
    full[:, 19, :, 0, 0] = v19[:, _S19[:, 0]]
    full[:, 19, :, 0, 1] = v19[:, _S19[:, 1]]
    return full


# revision 33
# speedup vs baseline: 3.5328x; 1.0003x over previous
"""Trainium2 Bass kernel for the batched MPS quantum-circuit forward pass.

Polar-form closed solution. Each gate is U_l = RZ(z)RY(y); the chained site
tensor entry is a product of 4 gate entries g_l(row_l, col_l) with
row_l = a_l ^ m_l, col_l = a_{l-1}.  Writing each entry as
(+-cos/sin(y_l/2)) * e^{+-i z_l/2} (phase sign = row bit), every output
element is  Mg * e^{i phi}  where Mg (signed magnitude) is a product of 4
real table entries and phi = sum_l (2*row_l - 1) * z_l/2 depends only on the
row bits t = (t0t1t2t3).

In t-coordinates (t_l = a_l ^ m_l) the value is independent of m3 and of the
rho-placement, so an interior site has only 8(m0m1m2) x 16(t) = 128 unique
complex values; the XOR placement rho = t ^ lam is a static permutation done
on the host.  Per site the device computes:
    Mg[m012, t] = M01x[m0, t0, t1] * M23[m1m2, t1t2t3]    (128 muls)
    im = Mg * sinT[t],  re = Mg * cosT[t]                 (2 x 128 muls)
with sinT/cosT built from a range-reduced phase table (phi/2pi - round, via
the f16 magic-constant trick) and one Sin activation each.

Output is compact f16: 18 interior sites x 256 (im|re planes) + site0 (16
complex, lam=0 only) + site19 (16 complex: sum over t0t1t2, index (m012,t3)).
The host expands to the full (1024,20,16,16,2) complex64 tensor (structural
zeros + XOR reindex); no host arithmetic beyond re+i*im.

Sharding: pure data parallelism, batch 1024 = 8 cores x 128 partitions.
All tensor instructions use <=3 free dims (hw TENSOR3D limit).
"""

import sys

sys.path.insert(0, "/opt/trn_rl_repo")

import numpy as np

B_TOTAL = 1024
N_CORES = 8
B = B_TOTAL // N_CORES
NQ = 20
P_COLS = 160
OUT_COLS = 4672  # 3*1536 interior + 32 site0 + 32 site19

_CACHE = {}


def _build_nc():
    import concourse.bass as bass
    import concourse.tile as tile
    from concourse import bacc, mybir

    f32 = mybir.dt.float32
    f16 = mybir.dt.float16
    ALU = mybir.AluOpType
    MUL = ALU.mult
    SIN = mybir.ActivationFunctionType.Sin
    ABS = mybir.ActivationFunctionType.Abs

    TWO_PI = float(2 * np.pi)
    INV_4PI = float(1.0 / (4 * np.pi))
    # Round-to-int magic for the f32 ALU datapath (storage dtype is f16 but
    # the DVE computes in f32, so the f32 magic is the one that rounds).
    MAGIC = 12582912.0  # 1.5 * 2^23

    nc = bacc.Bacc("TRN2", target_bir_lowering=False, debug=False)
    theta_d = nc.dram_tensor("theta", [B, P_COLS], f32, kind="ExternalInput").ap()
    out_d = nc.dram_tensor("out", [B, OUT_COLS], f16, kind="ExternalOutput").ap()

    from contextlib import ExitStack

    with tile.TileContext(nc) as tc, ExitStack() as ctx:
        pool = ctx.enter_context(tc.tile_pool(name="main", bufs=1))

        def tl(name, w, dt=f16):
            return pool.tile([B, w], dt, name=name)

        th = tl("th", 160, f32)
        zs = tl("zs", 80)           # z/(4pi) f16 (l*20+q)
        pq = tl("pq", 160)          # q*8 + [P(t0t1):0..3 | Q(t2t3):4..7]
        phs = tl("phs", 320)        # q*16 + ts, ts = t0*8+t1*4+t2*2+t3
        kk = tl("kk", 320)          # round(phs)
        ww = tl("ww", 320)          # phs - round(phs) in [-.5,.5]
        nw = tl("nw", 320)
        aw = tl("aw", 320)
        sinT = tl("sinT", 320)      # sin(phi)  (plane 0 = im)
        cosT = tl("cosT", 320)      # cos(phi)
        gt = tl("gt", 320)          # (l*4 + row*2 + col)*20 + q
        m01x = tl("m01x", 640)      # q*32 + m0*16 + t0*8 + t1*4 + t2t3(dup)
        m23 = tl("m23", 640)        # q*32 + (m1*2+m2)*8 + t1*4 + t2*2 + t3
        mg = tl("mg", 2560)         # q*128 + m012*16 + ts
        outt = tl("outt", OUT_COLS)  # g*1536 + plane*768 + qg*128 + m012*16 + ts
        p19s = tl("p19s", 128)
        p19c = tl("p19c", 128)
        r1s = tl("r1s", 64)
        r2s = tl("r2s", 32)
        r1c = tl("r1c", 64)
        r2c = tl("r2c", 32)
        halfpi = tl("halfpi", 1, f32)
        warm = tl("warm", 1, f32)

        def ap(t, off, dims):
            w = t.shape[1]
            return bass.AP(tensor=t.tensor, offset=t.offset + off, ap=[[w, B]] + dims)

        # ---- warm the Sin activation table before theta arrives ----------
        nc.vector.memset(halfpi[:], float(np.pi / 2))
        nc.scalar.activation(warm[:], halfpi[:], SIN, scale=0.5)
        nc.sync.dma_start(th[:], theta_d)

        # ---- Act: gate tables -------------------------------------------
        # cos(y/2) = Sin(pi/2 - y/2) directly; for the rare y < -pi the Sin
        # table arg exceeds pi where its error is still only ~1e-3 (measured).
        yl = [[40, 4], [1, 20]]
        nc.scalar.activation(
            ap(gt, 0, [[80, 4], [60, 2], [1, 20]]),
            ap(th, 0, [[40, 4], [0, 2], [1, 20]]),
            SIN, bias=halfpi[:], scale=-0.5,
        )
        nc.scalar.activation(ap(gt, 40, [[80, 4], [1, 20]]), ap(th, 0, yl), SIN, scale=0.5)
        nc.scalar.activation(ap(gt, 20, [[80, 4], [1, 20]]), ap(th, 0, yl), SIN, scale=-0.5)

        # ---- DVE: phase chain -------------------------------------------
        nc.vector.tensor_scalar_mul(
            ap(zs, 0, [[20, 4], [1, 20]]), ap(th, 20, yl), INV_4PI
        )
        zin = lambda o: ap(zs, o, [[1, 20], [40, 2]])
        nc.vector.tensor_add(ap(pq, 3, [[8, 20], [4, 2]]), zin(0), zin(20))
        nc.vector.tensor_sub(ap(pq, 2, [[8, 20], [4, 2]]), zin(0), zin(20))
        nc.vector.tensor_scalar_mul(
            ap(pq, 0, [[8, 20], [4, 2], [1, 2]]),
            ap(pq, 3, [[8, 20], [4, 2], [-1, 2]]),
            -1.0,
        )
        # phs[q, ts] = P[t0t1] + Q[t2t3]; per-t0 instr, iter [t1, t2t3, q]
        for t0 in (0, 1):
            nc.vector.tensor_add(
                ap(phs, t0 * 8, [[4, 2], [1, 4], [16, 20]]),
                ap(pq, t0 * 2, [[1, 2], [0, 4], [8, 20]]),
                ap(pq, 4, [[0, 2], [1, 4], [8, 20]]),
            )
        nc.vector.tensor_scalar(kk[:], phs[:], MAGIC, MAGIC, ALU.add, ALU.subtract)
        nc.vector.tensor_sub(ww[:], phs[:], kk[:])

        # ---- Act: phase trig (abs on Act: it is idle waiting for ww) -----
        nc.scalar.activation(aw[:], ww[:], ABS)
        nc.scalar.activation(sinT[:], ww[:], SIN, scale=TWO_PI)
        nc.scalar.activation(cosT[:], aw[:], SIN, bias=halfpi[:], scale=-TWO_PI)

        # ---- m01x on DVE (early: Pool's mgBC half depends on it) ---------
        # m01x[q, m0, t0, t1, dup t2t3] = g0(t0,0)*g1(t1, t0^m0); iter [q,t1,t2t3]
        with tc.high_priority():
            for m0 in (0, 1):
                for t0 in (0, 1):
                    nc.vector.tensor_tensor(
                        ap(m01x, m0 * 16 + t0 * 8, [[32, 20], [4, 2], [1, 4]]),
                        ap(gt, t0 * 40, [[1, 20], [0, 2], [0, 4]]),
                        ap(gt, 80 + ((t0 ^ m0) * 20), [[1, 20], [40, 2], [0, 4]]),
                        MUL,
                    )

        # ---- m23 on Pool (m1=0) + DVE (m1=1) -----------------------------
        # m23[q,m12,t1t2t3] = g2(t2, t1^m1) * g3(t3, t2^m2); iter [t2,t3,q]
        def m23_build(eng, m1, m2, t1):
            eng.tensor_tensor(
                ap(m23, (m1 * 2 + m2) * 8 + t1 * 4, [[2, 2], [1, 2], [32, 20]]),
                ap(gt, 160 + ((t1 ^ m1) * 20), [[40, 2], [0, 2], [1, 20]]),
                ap(gt, 240 + m2 * 20, [[20 - m2 * 40, 2], [40, 2], [1, 20]]),
                MUL,
            )

        for m1 in (0, 1):
            for m2 in (0, 1):
                for t1 in (0, 1):
                    eng = nc.vector if (m1, m2) == (1, 1) else nc.gpsimd
                    m23_build(eng, m1, m2, t1)

        # ---- mg: per (m0,t0), iter [q, m12, t123] ------------------------
        def mg_mul(q0, nq, m0, t0, eng=None):
            (eng or nc.vector).tensor_tensor(
                ap(mg, q0 * 128 + m0 * 64 + t0 * 8, [[128, nq], [16, 4], [1, 8]]),
                ap(m01x, q0 * 32 + m0 * 16 + t0 * 8, [[32, nq], [0, 4], [1, 8]]),
                ap(m23, q0 * 32, [[32, nq], [8, 4], [1, 8]]),
                MUL,
            )

        def finals(base, q0, nq, plane, T):
            nc.vector.tensor_tensor(
                ap(outt, base + plane * nq * 128, [[128, nq], [16, 8], [1, 16]]),
                ap(mg, q0 * 128, [[128, nq], [16, 8], [1, 16]]),
                ap(T, q0 * 16, [[16, nq], [0, 8], [1, 16]]),
                MUL,
            )

        # group A: q1..6 (site0 from q0 block)
        mg_mul(0, 7, 0, 0)
        mg_mul(0, 7, 0, 1)
        mg_mul(1, 6, 1, 0)
        mg_mul(1, 6, 1, 1)
        with tc.high_priority():
            finals(0, 1, 6, 0, sinT)
            nc.sync.dma_start(out_d[:, 0:768], outt[:, 0:768])
            finals(0, 1, 6, 1, cosT)
            nc.sync.dma_start(out_d[:, 768:1536], outt[:, 768:1536])

        # groups B+C: q7..19; m0=1 half on Pool (its idle window)
        mg_mul(7, 13, 1, 0, nc.gpsimd)
        mg_mul(7, 13, 1, 1, nc.gpsimd)
        mg_mul(7, 13, 0, 0)
        mg_mul(7, 13, 0, 1)

        # site 0 on Pool: outt[4608 + plane*16 + ts] = mg[q0, m012=0] * T[q0]
        nc.gpsimd.tensor_tensor(
            ap(outt, 4608, [[1, 16]]), ap(mg, 0, [[1, 16]]), ap(sinT, 0, [[1, 16]]), MUL
        )
        nc.gpsimd.tensor_tensor(
            ap(outt, 4624, [[1, 16]]), ap(mg, 0, [[1, 16]]), ap(cosT, 0, [[1, 16]]), MUL
        )
        # site 19 (Pool, early so the merged C-re DMA is not tail-blocked)
        nc.gpsimd.tensor_tensor(
            p19s[:], ap(mg, 19 * 128, [[16, 8], [1, 16]]),
            ap(sinT, 19 * 16, [[0, 8], [1, 16]]), MUL,
        )
        nc.gpsimd.tensor_tensor(
            p19c[:], ap(mg, 19 * 128, [[16, 8], [1, 16]]),
            ap(cosT, 19 * 16, [[0, 8], [1, 16]]), MUL,
        )
        # reduce t0 (stride 8), then t1 (stride 4), then t2 (stride 2), keep t3
        for p, r1, r2, oc in ((p19s, r1s, r2s, 4640), (p19c, r1c, r2c, 4656)):
            nc.gpsimd.tensor_add(
                ap(r1, 0, [[8, 8], [1, 8]]),
                ap(p, 0, [[16, 8], [1, 8]]), ap(p, 8, [[16, 8], [1, 8]]),
            )
            nc.gpsimd.tensor_add(
                ap(r2, 0, [[4, 8], [1, 4]]),
                ap(r1, 0, [[8, 8], [1, 4]]), ap(r1, 4, [[8, 8], [1, 4]]),
            )
            nc.gpsimd.tensor_add(
                ap(outt, oc, [[2, 8], [1, 2]]),
                ap(r2, 0, [[4, 8], [1, 2]]), ap(r2, 2, [[4, 8], [1, 2]]),
            )
        with tc.high_priority():
            finals(1536, 7, 8, 0, sinT)
            nc.sync.dma_start(out_d[:, 1536:2560], outt[:, 1536:2560])
            finals(1536, 7, 8, 1, cosT)
            nc.sync.dma_start(out_d[:, 2560:3584], outt[:, 2560:3584])
            finals(3584, 15, 4, 0, sinT)
            finals(3584, 15, 4, 1, cosT)
            # whole group C + site0 + site19 in one DMA
            nc.sync.dma_start(out_d[:, 3584:4672], outt[:, 3584:4672])

    nc.compile()
    return nc


def _get_nc():
    if "nc" not in _CACHE:
        _CACHE["nc"] = _build_nc()
    return _CACHE["nc"]


# host-side static index maps (natural ts order: ts == t)
_LAM = np.arange(16)
_S = (_LAM[:, None] >> 1) * 16 + (_LAM[:, None] ^ _LAM[None, :])
_S19 = (_LAM[:, None] >> 1) * 2 + (np.arange(2)[None, :] ^ (_LAM[:, None] & 1))


def kernel(theta, batch_size):
    from concourse.bass_utils import run_bass_kernel_spmd

    theta = np.ascontiguousarray(np.asarray(theta), dtype=np.float32)
    assert theta.shape == (B_TOTAL, P_COLS)
    nc = _get_nc()
    in_maps = [{"theta": theta[c * B : (c + 1) * B]} for c in range(N_CORES)]
    res = run_bass_kernel_spmd(nc, in_maps, core_ids=list(range(N_CORES)))
    _CACHE["last_res"] = res
    buf = np.concatenate([r["out"] for r in res.results], axis=0).astype(np.float32)

    full = np.zeros((B_TOTAL, NQ, 16, 16, 2), np.complex64)
    vs = []
    for base, nq in ((0, 6), (1536, 8), (3584, 4)):
        g = buf[:, base : base + nq * 256].reshape(B_TOTAL, 2, nq, 128)
        vs.append(g[:, 1] + 1j * g[:, 0])
    v = np.concatenate(vs, axis=1)  # [b, 18, m012*16+ts]
    fi = v[:, :, _S]  # [b, 18, lam, rho]
    full[:, 1:19, :, 0::2, 0] = fi[..., 0::2]
    full[:, 1:19, :, 1::2, 1] = fi[..., 1::2]
    s0 = buf[:, 4608:4640].reshape(B_TOTAL, 2, 16)
    v0 = s0[:, 1] + 1j * s0[:, 0]
    full[:, 0, 0, 0::2, 0] = v0[:, 0::2]
    full[:, 0, 0, 1::2, 1] = v0[:, 1::2]
    s19 = buf[:, 4640:4672].reshape(B_TOTAL, 2, 16)
    v19 = s19[:, 1] + 1j * s19[:, 0]  # [b, m012*2+t3]
    full[:, 19, :, 0, 0] = v19[:, _S19[:, 0]]
    full[:, 19, :, 0, 1] = v19[:, _S19[:, 1]]
    return full
